# revision 1
# baseline (speedup 1.0000x reference)
"""CRF loss (log_z - gold_score) on 8 Trainium2 NeuronCores.

Strategy (data-parallel over batch, per the sharding hint):
  - Shard the 1024-item batch as 128 contiguous items per core.
  - Device computes log_z via the forward algorithm in probability domain:
      a_t = (E^T a_{t-1}) * F_t,  E = exp(trans) (block-diag, PE-stationary),
      F_t = exp(emit_t) (ACT), one matmul + one DVE multiply per step.
  - Layout fold: a is [128 part = tag j + 64*h, 64 cols], i.e. two halves of
    64 batch items stacked on the partition axis -> full-width engines and a
    single K=128 matmul per step against blockdiag(E, E).
  - Numerical stability: every KR steps measure per-column sums s (PE with a
    block-diag ones stationary), take r = 1/s (DVE reciprocal), fold r into a
    later step's F (off the critical path), and account c = -sum ln(r) at the
    end (one batched ACT Ln over the stored factors).
  - Masking costs nothing on device: the host bakes it into the emissions.
    At each column's last real step t = len-1 we add (etrans - trans[:,0]);
    afterwards all tags except tag 0 get -1e30 (exp -> 0) and tag 0 gets
    0 (first pad) then -trans[0,0], which collapses the forward value into
    tag 0 and preserves it exactly until the common final step.
  - Final: logz_b = ln(a[tag0, b]) + c_b;  gold path score is O(L*B) gather
    bookkeeping done on the host;  loss = mean(logz) - mean(gold).
"""

import sys
from contextlib import ExitStack

import numpy as np

sys.path.insert(0, "/opt/trn_rl_repo")

import ml_dtypes  # noqa: E402
import concourse.bass as bass  # noqa: E402
import concourse.tile as tile  # noqa: E402
from concourse import bacc, mybir  # noqa: E402
from concourse.bass_utils import run_bass_kernel_spmd  # noqa: E402

BF16 = ml_dtypes.bfloat16

L, B, T, NC = 512, 1024, 64, 8
CH = 8                      # steps per DMA/exp chunk
TMAX = 520                  # padded step count (multiple of CH, >= L+1)
NCHUNK = TMAX // CH
KR, LAG = 10, 3             # renorm cadence / apply lag
RENORM_TS = [t for t in range(KR, TMAX - LAG - 1, KR)]
NR = len(RENORM_TS)
NEG = np.float32(-1e30)

_CACHE = {}


def _build_nc(tmax=TMAX, renorm_ts=None, streams=2, reps=1):
    if renorm_ts is None:
        renorm_ts = [t for t in range(KR, tmax - LAG - 1, KR)]
    nr = len(renorm_ts)
    cw = 64 // streams  # columns per stream
    f32 = mybir.dt.float32
    bf = mybir.dt.bfloat16
    nc = bacc.Bacc("TRN2", target_bir_lowering=False, debug=False)
    emitf_d = nc.dram_tensor("emitf", [NCHUNK, 128, CH * 64], bf, kind="ExternalInput")
    e2_d = nc.dram_tensor("e2", [128, 128], bf, kind="ExternalInput")
    es2_d = nc.dram_tensor("es2", [128, 1], f32, kind="ExternalInput")
    onesbd_d = nc.dram_tensor("onesbd", [128, 2], bf, kind="ExternalInput")
    halfsel_d = nc.dram_tensor("halfsel", [2, 128], bf, kind="ExternalInput")
    sel0_d = nc.dram_tensor("sel0", [128, 2], bf, kind="ExternalInput")
    fin_d = nc.dram_tensor("fin", [2, 64], f32, kind="ExternalOutput")
    kbuf_d = nc.dram_tensor("kbuf", [2, 64, nr], f32, kind="ExternalOutput")

    with tile.TileContext(nc) as tc, ExitStack() as ctx:
        cpool = ctx.enter_context(tc.tile_pool(name="consts", bufs=1))
        epool = ctx.enter_context(tc.tile_pool(name="emit", bufs=6))
        fpool = ctx.enter_context(tc.tile_pool(name="fexp", bufs=6))
        small = ctx.enter_context(tc.tile_pool(name="small", bufs=4))
        upsum = ctx.enter_context(tc.tile_pool(name="upsum", bufs=2, space="PSUM"))
        spsum = ctx.enter_context(tc.tile_pool(name="spsum", bufs=2, space="PSUM"))
        kpsum = ctx.enter_context(tc.tile_pool(name="kpsum", bufs=2, space="PSUM"))

        E2sb = cpool.tile([128, 128], bf, tag="E2sb")
        nc.sync.dma_start(E2sb[:], e2_d[:])
        es2sb = cpool.tile([128, 1], f32, tag="es2sb")
        nc.sync.dma_start(es2sb[:], es2_d[:])

        onesbd = cpool.tile([128, 2], bf, tag="onesbd")
        nc.sync.dma_start(onesbd[:], onesbd_d[:])
        halfsel = cpool.tile([2, 128], bf, tag="halfsel")
        nc.sync.dma_start(halfsel[:], halfsel_d[:])
        sel0 = cpool.tile([128, 2], bf, tag="sel0")
        nc.sync.dma_start(sel0[:], sel0_d[:])

        Kbuf = cpool.tile([2, 64, nr], f32, tag="Kbuf")
        atile = cpool.tile([128, 64], bf, tag="atile")

        def load_chunk(ci):
            et = epool.tile([128, CH * 64], bf, tag="et")
            # alternate issuing engines so chunk loads land on two DMA
            # queues instead of serializing on one
            eng = nc.sync if ci % 2 == 0 else nc.gpsimd
            eng.dma_start(et[:], emitf_d[ci])
            ft = fpool.tile([128, CH * 64], bf, tag="ft")
            nc.scalar.activation(ft[:], et[:], mybir.ActivationFunctionType.Exp)
            return ft

        apply_at = {tm + LAG: r for r, tm in enumerate(renorm_ts)}
        renorm_set = set(renorm_ts)

        for _rep in range(reps):
          kexp_sb = {}
          fch = load_chunk(0)
          # a_0 = exp(strans) * F_0
          nc.vector.tensor_scalar(
              atile[:], fch[:, 0:64], es2sb[:, 0:1], None, mybir.AluOpType.mult
          )

          for t in range(1, tmax):
            ci, s = divmod(t, CH)
            if s == 0:
                fch = load_chunk(ci)
            Fs = fch[:, 64 * s : 64 * (s + 1)]

            if t in apply_at:
                r = apply_at[t]
                kt = kpsum.tile([128, 64], mybir.dt.float32, tag="kt")
                nc.tensor.matmul(kt[:], halfsel[:], kexp_sb[r][:])
                nc.vector.tensor_tensor(Fs, Fs, kt[:], mybir.AluOpType.mult)

            # independent per-column-slice chains; interleaving lets the PE
            # run stream s+1's matmul while the DVE multiplies stream s
            for s_ in range(streams):
                cs = slice(cw * s_, cw * (s_ + 1))
                fs = slice(64 * s + cw * s_, 64 * s + cw * (s_ + 1))
                u = upsum.tile([128, cw], mybir.dt.float32, tag=f"u{s_}")
                nc.tensor.matmul(u[:], E2sb[:], atile[:, cs])
                nc.vector.tensor_tensor(
                    atile[:, cs], u[:], fch[:, fs], mybir.AluOpType.mult
                )

            if t in renorm_set:
                r = renorm_ts.index(t)
                sp = spsum.tile([2, 64], mybir.dt.float32, tag="sp")
                nc.tensor.matmul(sp[:], onesbd[:], atile[:])
                rec = small.tile([2, 64], mybir.dt.float32, tag="rec")
                nc.vector.reciprocal(rec[:], sp[:])
                kb = small.tile([2, 64], mybir.dt.bfloat16, tag="kb")
                nc.vector.tensor_copy(kb[:], rec[:])
                # store the exact applied (bf16) value, upcast, for Ln later
                nc.vector.tensor_copy(Kbuf[:, :, r], kb[:])
                kexp_sb[r] = kb

        # final: host computes logz = ln(fin) - sum_r ln(Kbuf) in f64;
        # device ships the raw values (ACT Ln is inaccurate on tiny inputs).
        fin = spsum.tile([2, 64], mybir.dt.float32, tag="sp")
        nc.tensor.matmul(fin[:], sel0[:], atile[:])
        finsb = small.tile([2, 64], mybir.dt.float32, tag="finsb")
        nc.vector.tensor_copy(finsb[:], fin[:])
        nc.sync.dma_start(fin_d[:], finsb[:])
        nc.sync.dma_start(kbuf_d[:], Kbuf[:])

    nc.compile()
    return nc


def _prepare_host(emit, trans, strans, etrans, mask):
    lens = mask.sum(0).astype(np.int64)  # [B], all >= 1 (mask[0] all True)
    ar = np.arange(B)
    emitP = np.empty((TMAX, B, T), np.float32)
    emitP[:L] = emit
    emitP[L:] = NEG
    # fold end transition into the last real step
    emitP[lens - 1, ar, :] += (etrans - trans[:, 0])[None, :]
    # pad steps: -inf except tag 0
    tgrid = np.arange(TMAX)[:, None]
    padmask = tgrid >= lens[None, :]  # [TMAX, B]
    emitP[padmask] = NEG
    emitP[lens, ar, 0] = 0.0  # first pad step collapses into tag 0
    laterpad = tgrid > lens[None, :]
    e0 = emitP[:, :, 0]
    e0[laterpad] = -trans[0, 0]

    E = np.exp(trans.astype(np.float32))
    E2 = np.zeros((128, 128), np.float32)
    E2[:64, :64] = E
    E2[64:, 64:] = E
    E2 = E2.astype(BF16)
    es2 = np.concatenate([np.exp(strans)] * 2).astype(np.float32).reshape(128, 1)

    onesbd = np.zeros((128, 2), np.float32)
    onesbd[:64, 0] = 1.0
    onesbd[64:, 1] = 1.0
    onesbd = onesbd.astype(BF16)
    halfsel = np.zeros((2, 128), np.float32)
    halfsel[0, :64] = 1.0
    halfsel[1, 64:] = 1.0
    halfsel = halfsel.astype(BF16)
    sel0 = np.zeros((128, 2), np.float32)
    sel0[0, 0] = 1.0
    sel0[64, 1] = 1.0
    sel0 = sel0.astype(BF16)

    in_maps = []
    for c in range(NC):
        ec = emitP[:, 128 * c : 128 * (c + 1), :]  # [TMAX, 128, 64] (b_local, j)
        v = ec.reshape(TMAX, 2, 64, T)  # [t, h, b', j]
        emitF = np.ascontiguousarray(v.transpose(0, 1, 3, 2)).reshape(TMAX, 128, 64)
        emitf_np = np.ascontiguousarray(
            emitF.reshape(NCHUNK, CH, 128, 64).transpose(0, 2, 1, 3)
        ).reshape(NCHUNK, 128, CH * 64).astype(BF16)
        in_maps.append({
            "emitf": emitf_np, "e2": E2, "es2": es2,
            "onesbd": onesbd, "halfsel": halfsel, "sel0": sel0,
        })
    return in_maps, lens


def _gold_score(emit, trans, strans, etrans, target, mask, lens):
    target = target.astype(np.int64)
    emit_sc = np.take_along_axis(emit, target[:, :, None], axis=2)[..., 0]
    trans_sc = np.concatenate(
        [np.zeros((1, B), np.float32), trans[target[:-1], target[1:]]], axis=0
    )
    score = np.where(mask, emit_sc + trans_sc, np.float32(0.0)).sum(dtype=np.float32)
    score = score + strans[target[0]].sum(dtype=np.float32)
    last_tag = target[lens - 1, np.arange(B)]
    score = score + etrans[last_tag].sum(dtype=np.float32)
    return score / np.float32(B)


def kernel(emit, trans, strans, etrans, target, mask):
    emit = np.asarray(emit, np.float32)
    trans = np.asarray(trans, np.float32)
    strans = np.asarray(strans, np.float32)
    etrans = np.asarray(etrans, np.float32)
    mask_b = np.asarray(mask).astype(bool)

    in_maps, lens = _prepare_host(emit, trans, strans, etrans, mask_b)

    if "nc" not in _CACHE:
        _CACHE["nc"] = _build_nc()
    nc = _CACHE["nc"]
    res = run_bass_kernel_spmd(nc, in_maps, core_ids=list(range(NC)))

    logz = np.empty(B, np.float64)
    for c in range(NC):
        fin = np.asarray(res.results[c]["fin"], np.float64)  # [2, 64]
        kbuf = np.asarray(res.results[c]["kbuf"], np.float64)  # [2, 64, NR]
        o = np.log(fin) - np.log(kbuf).sum(-1)
        for h in range(2):
            logz[128 * c + 64 * h : 128 * c + 64 * h + 64] = o[h]
    log_z = np.float32(logz.sum() / B)

    gold = _gold_score(emit, trans, strans, etrans, np.asarray(target), mask_b, lens)
    return np.asarray(log_z - gold, dtype=np.float32)

